# revision 40
# baseline (speedup 1.0000x reference)
"""Neural SDE Euler integrator on 8 Trainium2 NeuronCores.

Data-parallel: 4096 SDE rows split 512/core; MLP params replicated.
On-chip layout is feature-major [feat, rows] so every MLP layer is a
stationary-weight matmul with rows (512) as the moving free dim.

Drift-MLP matmuls run in bf16 (fp32 PSUM accumulate); the SDE state is
kept in fp32 and only its bf16 copy feeds the MLP. The logqp
cross-partition reduction runs in float32r. Rank-1 terms are folded
into the big matmuls: the treatment-MLP contribution V[s] (x) Tcol rides
as an extra contraction row of layer 1, and the layer-6 bias rides as a
ones-row on the last activation chunk.

Self-contained: hardcodes all shapes; call kernel(**inputs) with the
full unsharded inputs, returns (ys, logqp) like the reference.
"""

import numpy as np
import ml_dtypes

import concourse.bass as bass
import concourse.mybir as mybir
import concourse.tile as tile
from concourse import bacc
from concourse import bass_utils

NCORES = 8
LATENT = 100
U_DIM = 20
NUM_SAMPLES = 8
BATCH = 512
ROWS = (BATCH * NUM_SAMPLES) // NCORES  # 512 rows per core
HID = 400
N_STEPS = 50
T_LEN = N_STEPS + 1
DT = 0.05
SIGMA = 0.5
SQDT_SIG = float(np.sqrt(DT) * SIGMA)  # 0.1118034

F32 = mybir.dt.float32
F32R = mybir.dt.float32r
BF16 = mybir.dt.bfloat16
ADD = mybir.AluOpType.add
MULT = mybir.AluOpType.mult
MAX = mybir.AluOpType.max
Act = mybir.ActivationFunctionType

# 400-wide hidden dim chunked onto <=128 partitions
CH = [(0, 128), (128, 128), (256, 128), (384, 16)]

TRACE = False
LAST_EXEC_NS = None
_CACHE = {}


def _build(n_steps=N_STEPS):
    nc = bacc.Bacc("TRN2", target_bir_lowering=False, debug=False,
                   num_devices=NCORES)

    def din(name, shape, dt=F32):
        return nc.dram_tensor(name, shape, dt, kind="ExternalInput").ap()

    z0f = din("z0f", [LATENT, ROWS])
    nzd = din("noise", [n_steps, LATENT, ROWS])
    tcol_d = din("tcol", [1, ROWS], BF16)
    trow_d = din("trow", [1, n_steps], BF16)
    tw_d = [din(f"tw{i}", [1 if i == 0 else U_DIM, U_DIM], BF16)
            for i in range(6)]
    tb_d = [din(f"tb{i}", [U_DIM, 1]) for i in range(6)]
    w1z_d = din("w1z", [LATENT, HID], BF16)
    w1u_d = din("w1u", [U_DIM, HID], BF16)
    b1_d = din("b1", [HID, 1])
    wmid_d = [din(f"w{l}", [HID, HID], BF16) for l in (2, 3, 4, 5)]
    bmid_d = [din(f"b{l}", [HID, 1]) for l in (2, 3, 4, 5)]
    w6a_d = din("w6a", [HID + 1, LATENT], BF16)  # row 400 = b6
    onesred_d = din("onesred", [LATENT, 1])      # 0.1 each
    onesrow_d = din("onesrow", [1, ROWS], BF16)  # 1.0
    traj = nc.dram_tensor("traj", [n_steps, LATENT, ROWS], F32,
                          kind="ExternalOutput").ap()
    # per-step logqp increments 0.1*sum_d q^2; host cumsums over steps
    lq_red = nc.dram_tensor("lq_red", [n_steps, ROWS], F32,
                            kind="ExternalOutput").ap()

    # layer-6 contraction chunks over 401 rows (last chunk carries b6 row)
    CH6 = [(0, 128), (128, 128), (256, 128), (384, 17)]

    with tile.TileContext(nc) as tc:
        with tc.tile_pool(name="const", bufs=1) as const, \
             tc.tile_pool(name="acts", bufs=12) as acts, \
             tc.tile_pool(name="states", bufs=3) as states, \
             tc.tile_pool(name="tmps", bufs=3) as tmps, \
             tc.tile_pool(name="noisep", bufs=3) as noisep, \
             tc.tile_pool(name="psL", bufs=6, space="PSUM") as psL, \
             tc.tile_pool(name="psF", bufs=1, space="PSUM") as psF, \
             tc.tile_pool(name="psR", bufs=1, space="PSUM") as psR, \
             tc.tile_pool(name="dramp", bufs=1, space="DRAM") as dramp:

            def load(ap, p, f, tag, dt=BF16, eng=None):
                t = const.tile([p, f], dt, tag=tag)
                (eng or nc.sync).dma_start(out=t, in_=ap)
                return t

            # ---- weights / constants (DMA'd straight in as bf16) ----
            # treat-MLP inputs + W1 first (they gate step 0); the bulky
            # mid-layer weights ride the gpsimd DMA queue in parallel
            tw = [load(tw_d[i], 1 if i == 0 else U_DIM, U_DIM, f"tw{i}")
                  for i in range(6)]
            tb = [load(tb_d[i], U_DIM, 1, f"tb{i}", F32) for i in range(6)]
            trow = load(trow_d, 1, n_steps, "trow")
            w1u = load(w1u_d, U_DIM, HID, "w1u")
            # W1 augmented: rows 0:100 = W1z (static), row 100 = V[s] per step
            w1a = const.tile([LATENT + 1, HID], BF16, tag="w1a")
            nc.sync.dma_start(out=w1a[0:LATENT, :], in_=w1z_d)
            b1 = [load(b1_d[m0:m0 + mc, :], mc, 1, f"b1_{mi}", F32)
                  for mi, (m0, mc) in enumerate(CH)]
            # logqp reduction vector in f32r
            onesred_st = const.tile([LATENT, 1], F32, tag="onesred_st")
            nc.sync.dma_start(out=onesred_st, in_=onesred_d)
            onesred = const.tile([LATENT, 1], F32R, tag="onesred")
            nc.vector.tensor_copy(onesred, onesred_st)
            wmid = [[load(wd[k0:k0 + kc, :], kc, HID, f"w{l}_{ki}",
                          eng=nc.gpsimd)
                     for ki, (k0, kc) in enumerate(CH)]
                    for l, wd in zip((2, 3, 4, 5), wmid_d)]
            bmid = [[load(bd[m0:m0 + mc, :], mc, 1, f"b{l}_{mi}", F32,
                          eng=nc.gpsimd)
                     for mi, (m0, mc) in enumerate(CH)]
                    for l, bd in zip((2, 3, 4, 5), bmid_d)]
            w6 = [load(w6a_d[k0:k0 + kc, :], kc, LATENT, f"w6_{ki}",
                       eng=nc.gpsimd)
                  for ki, (k0, kc) in enumerate(CH6)]

            # ---- treat MLP for all steps at once: h [20, n_steps] ----
            h = trow
            for i in range(6):
                p = psR.tile([U_DIM, n_steps], F32, tag="pr")
                nc.tensor.matmul(p, tw[i], h, start=True, stop=True)
                hn = const.tile([U_DIM, n_steps], BF16, tag=f"th{i}")
                nc.scalar.activation(hn, p, Act.Relu if i < 5 else Act.Identity,
                                     bias=tb[i])
                h = hn

            # V[s,:] = u_all[:,s] @ W1u  -> [n_steps, HID] rows, then bounce
            # through DRAM so each V[s] can be DMA'd onto w1a row 100.
            pV = psL.tile([n_steps, HID], F32, tag="pl")
            nc.tensor.matmul(pV, h, w1u, start=True, stop=True)
            vsb = const.tile([n_steps, HID], BF16, tag="vsb")
            nc.scalar.copy(vsb, pV)
            vdram = dramp.tile([n_steps, HID], BF16)
            nc.sync.dma_start(out=vdram, in_=vsb)

            # ---- initial state ----
            z = states.tile([LATENT, ROWS], F32, tag="z")
            nc.sync.dma_start(out=z, in_=z0f)
            # per-step logqp increments parked here, one DMA at the end
            redall = const.tile([1, n_steps * ROWS], F32, tag="redall")
            # fixed bf16 MLP-input tile: rows 0:100 = z (rewritten per
            # step), row 100 = Tcol (written once)
            za = const.tile([LATENT + 1, ROWS], BF16, tag="za")
            nc.sync.dma_start(out=za[LATENT:LATENT + 1, :], in_=tcol_d)
            nc.vector.tensor_copy(za[0:LATENT, :], z)
            # fixed last-chunk tile of act5: row 16 = ones (carries b6)
            act5c3 = const.tile([17, ROWS], BF16, tag="act5c3")
            nc.sync.dma_start(out=act5c3[16:17, :], in_=onesrow_d)

            # ---- Euler scan ----
            for s in range(n_steps):
                nt = noisep.tile([LATENT, ROWS], F32, tag="noise")
                nc.sync.dma_start(out=nt, in_=nzd[s])
                # V[s] onto the augmented-W1 contraction row
                nc.sync.dma_start(out=w1a[LATENT:LATENT + 1, :],
                                  in_=vdram[s:s + 1, :])

                # layer 1: relu(W1z.T@z + V[s] x Tcol + b1); row-halved
                # so it can start as soon as za's first half is updated
                HR = ROWS // 2
                pl1 = [psL.tile([mc, ROWS], F32, tag="pl", name=f"pl1_{mi}")
                       for mi, (m0, mc) in enumerate(CH)]
                for h in range(2):
                    for mi, (m0, mc) in enumerate(CH):
                        nc.tensor.matmul(pl1[mi][:, h * HR:(h + 1) * HR],
                                         w1a[:, m0:m0 + mc],
                                         za[:, h * HR:(h + 1) * HR],
                                         start=True, stop=True,
                                         skip_group_check=True)
                prev = []
                for mi, (m0, mc) in enumerate(CH):
                    t = acts.tile([mc, ROWS], BF16, tag="act")
                    nc.vector.tensor_scalar(t, pl1[mi], b1[mi], 0.0, ADD, MAX)
                    prev.append(t)

                # layers 2-5: tanh(W.T @ x + b)
                for li in range(4):
                    last = li == 3
                    cur = []
                    for mi, (m0, mc) in enumerate(CH):
                        p = psL.tile([mc, ROWS], F32, tag="pl")
                        for ki, (k0, kc) in enumerate(CH):
                            nc.tensor.matmul(p, wmid[li][ki][:, m0:m0 + mc],
                                             prev[ki], start=(ki == 0),
                                             stop=(ki == 3))
                        # last chunk of act5 lives in the fixed ones-row tile
                        t = act5c3 if (last and mi == 3) else \
                            acts.tile([mc, ROWS], BF16, tag="act")
                        nc.scalar.activation(t[0:mc, :], p, Act.Tanh,
                                             bias=bmid[li][mi])
                        cur.append(t)
                    prev = cur

                # layer 6 (+b6 via ones row): pf = drift
                pf = psF.tile([LATENT, ROWS], F32, tag="pf")
                for ki in range(4):
                    nc.tensor.matmul(pf, w6[ki], prev[ki],
                                     start=(ki == 0), stop=(ki == 3))

                # s1 = 1.005 z (independent of drift; scheduler hoists it)
                s1 = tmps.tile([LATENT, ROWS], F32, tag="s1")
                nc.vector.scalar_tensor_tensor(s1, z, 0.005, z, MULT, ADD)
                # z_new = s1 + 0.05 drift + sig*sqrt(dt) dW; bf16 copy (za)
                # comes first, row-halved, since next-step layer 1 only
                # needs za's first half to start
                s2 = tmps.tile([LATENT, ROWS], F32, tag="s2")
                for h in range(2):
                    hs = slice(h * HR, (h + 1) * HR)
                    nc.vector.scalar_tensor_tensor(s2[:, hs], pf[:, hs], DT,
                                                   s1[:, hs], MULT, ADD)
                    nc.vector.scalar_tensor_tensor(za[0:LATENT, hs],
                                                   nt[:, hs], SQDT_SIG,
                                                   s2[:, hs], MULT, ADD)
                # q = drift + 0.2 z ; sq = q^2 (ACT) ; logqp increment
                q = tmps.tile([LATENT, ROWS], F32, tag="q")
                nc.vector.scalar_tensor_tensor(q, z, 0.2, pf, MULT, ADD)
                sq = tmps.tile([LATENT, ROWS], F32R, tag="sq")
                nc.scalar.square(sq, q)
                pr = psR.tile([1, ROWS], F32, tag="pr")
                nc.tensor.matmul(pr, onesred, sq, start=True, stop=True)
                nc.vector.tensor_copy(redall[:, s * ROWS:(s + 1) * ROWS], pr)
                znew = states.tile([LATENT, ROWS], F32, tag="z")
                nc.vector.scalar_tensor_tensor(znew, nt, SQDT_SIG, s2,
                                               MULT, ADD)
                nc.gpsimd.dma_start(out=traj[s, :, :], in_=znew)
                z = znew

            # ship all logqp increments out in one DMA
            nc.sync.dma_start(
                out=lq_red.rearrange("s r -> (s r)")[None, :], in_=redall)

    nc.compile()
    return nc


def _bf(x):
    return np.ascontiguousarray(x.astype(ml_dtypes.bfloat16))


def _prep_inputs(z0, t, Tx, noise, treat_params, drift_params):
    z0 = np.asarray(z0, np.float32)
    t = np.asarray(t, np.float32)
    Tx = np.asarray(Tx, np.float32)
    noise = np.asarray(noise, np.float32)
    tp = [(np.asarray(W, np.float32), np.asarray(b, np.float32))
          for W, b in treat_params]
    dp = [(np.asarray(W, np.float32), np.asarray(b, np.float32))
          for W, b in drift_params]

    z0_flat = np.ascontiguousarray(z0.reshape(-1, LATENT))  # [4096, 100]
    w6a = np.concatenate([dp[5][0], dp[5][1][None, :]], axis=0)  # [401, 100]
    shared = {
        "trow": _bf(t[:N_STEPS][None, :]),
        "onesred": np.full((LATENT, 1), 0.1, np.float32),
        "onesrow": _bf(np.ones((1, ROWS), np.float32)),
        "w1z": _bf(dp[0][0][:LATENT]),
        "w1u": _bf(dp[0][0][LATENT:]),
        "b1": np.ascontiguousarray(dp[0][1][:, None]),
        "w6a": _bf(w6a),
    }
    for i, l in enumerate((2, 3, 4, 5)):
        shared[f"w{l}"] = _bf(dp[i + 1][0])
        shared[f"b{l}"] = np.ascontiguousarray(dp[i + 1][1][:, None])
    for i in range(6):
        shared[f"tw{i}"] = _bf(tp[i][0])
        shared[f"tb{i}"] = np.ascontiguousarray(tp[i][1][:, None])

    in_maps = []
    bpc = BATCH // NCORES  # batches per core
    for c in range(NCORES):
        r0, r1 = c * ROWS, (c + 1) * ROWS
        m = dict(shared)
        m["z0f"] = np.ascontiguousarray(z0_flat[r0:r1].T)
        m["noise"] = np.ascontiguousarray(
            noise[:, r0:r1, :LATENT].transpose(0, 2, 1))
        m["tcol"] = _bf(np.repeat(Tx[c * bpc:(c + 1) * bpc],
                                  NUM_SAMPLES)[None, :])
        in_maps.append(m)
    return in_maps, z0_flat


def kernel(z0, t, Tx, noise, treat_params, drift_params):
    global LAST_EXEC_NS
    if "nc" not in _CACHE:
        _CACHE["nc"] = _build()
    nc = _CACHE["nc"]

    in_maps, z0_flat = _prep_inputs(z0, t, Tx, noise, treat_params,
                                    drift_params)
    try:
        res = bass_utils.run_bass_kernel_spmd(
            nc, in_maps, core_ids=list(range(NCORES)), trace=TRACE)
    except Exception:
        if not TRACE:
            raise
        res = bass_utils.run_bass_kernel_spmd(
            nc, in_maps, core_ids=list(range(NCORES)), trace=False)
    LAST_EXEC_NS = res.exec_time_ns

    # assemble [51, 4096, 101] then raw-reshape like the reference
    ntotal = BATCH * NUM_SAMPLES
    full = np.empty((T_LEN, ntotal, LATENT + 1), np.float32)
    full[0, :, :LATENT] = z0_flat
    full[0, :, LATENT] = 0.0
    for c in range(NCORES):
        r0, r1 = c * ROWS, (c + 1) * ROWS
        tr = res.results[c]["traj"]  # [n_steps, 100, 512]
        full[1:, r0:r1, :LATENT] = tr.transpose(0, 2, 1)
        # logqp state at time t = cumsum of the per-step increments
        red = res.results[c]["lq_red"]  # [n_steps, 512]
        full[1:, r0:r1, LATENT] = np.cumsum(red, axis=0)
    arr4 = full.reshape(BATCH, NUM_SAMPLES, T_LEN, LATENT + 1)
    ys = np.ascontiguousarray(arr4[:, :, :, :LATENT])
    logqp = np.ascontiguousarray(arr4[:, :, -1, LATENT])
    return ys, logqp


# revision 47
# speedup vs baseline: 1.1210x; 1.1210x over previous
"""Neural SDE Euler integrator on 8 Trainium2 NeuronCores.

Data-parallel: 4096 SDE rows split 512/core; MLP params replicated.
On-chip layout is feature-major [feat, rows] so every MLP layer is a
stationary-weight matmul with rows (512) as the moving free dim.

Drift-MLP matmuls run in bf16 (fp32 PSUM accumulate); the SDE state is
kept in fp32 and only its bf16 copy feeds the MLP. The logqp
cross-partition reduction runs in float32r. Rank-1 terms are folded
into the big matmuls: the treatment-MLP contribution V[s] (x) Tcol rides
as an extra contraction row of layer 1, and the layer-6 bias rides as a
ones-row on the last activation chunk.

Self-contained: hardcodes all shapes; call kernel(**inputs) with the
full unsharded inputs, returns (ys, logqp) like the reference.
"""

import numpy as np
import ml_dtypes

import concourse.bass as bass
import concourse.mybir as mybir
import concourse.tile as tile
from concourse import bacc
from concourse import bass_utils

NCORES = 8
LATENT = 100
U_DIM = 20
NUM_SAMPLES = 8
BATCH = 512
ROWS = (BATCH * NUM_SAMPLES) // NCORES  # 512 rows per core
HID = 400
N_STEPS = 50
T_LEN = N_STEPS + 1
DT = 0.05
SIGMA = 0.5
SQDT_SIG = float(np.sqrt(DT) * SIGMA)  # 0.1118034

F32 = mybir.dt.float32
F32R = mybir.dt.float32r
BF16 = mybir.dt.bfloat16
ADD = mybir.AluOpType.add
MULT = mybir.AluOpType.mult
MAX = mybir.AluOpType.max
Act = mybir.ActivationFunctionType

# 400-wide hidden dim chunked onto <=128 partitions
CH = [(0, 128), (128, 128), (256, 128), (384, 16)]

TRACE = False
LAST_EXEC_NS = None
_CACHE = {}


def _build(n_steps=N_STEPS):
    nc = bacc.Bacc("TRN2", target_bir_lowering=False, debug=False,
                   num_devices=NCORES)

    def din(name, shape, dt=F32):
        return nc.dram_tensor(name, shape, dt, kind="ExternalInput").ap()

    z0f = din("z0f", [LATENT, ROWS])
    nzd = din("noise", [n_steps, LATENT, ROWS])
    tcol_d = din("tcol", [1, ROWS], BF16)
    trow_d = din("trow", [1, n_steps], BF16)
    tw_d = [din(f"tw{i}", [1 if i == 0 else U_DIM, U_DIM], BF16)
            for i in range(6)]
    tb_d = [din(f"tb{i}", [U_DIM, 1]) for i in range(6)]
    w1z_d = din("w1z", [LATENT, HID], BF16)
    w1u_d = din("w1u", [U_DIM, HID], BF16)
    b1_d = din("b1", [HID, 1])
    wmid_d = [din(f"w{l}", [HID, HID], BF16) for l in (2, 3, 4, 5)]
    bmid_d = [din(f"b{l}", [HID, 1]) for l in (2, 3, 4, 5)]
    w6a_d = din("w6a", [HID + 1, LATENT], BF16)  # row 400 = b6
    onesred_d = din("onesred", [LATENT, 1])      # 0.1 each
    onesrow_d = din("onesrow", [1, ROWS], BF16)  # 1.0
    traj = nc.dram_tensor("traj", [n_steps, LATENT, ROWS], F32,
                          kind="ExternalOutput").ap()
    # per-step logqp increments 0.1*sum_d q^2; host cumsums over steps
    lq_red = nc.dram_tensor("lq_red", [n_steps, ROWS], F32,
                            kind="ExternalOutput").ap()

    # layer-6 contraction chunks over 401 rows (last chunk carries b6 row)
    CH6 = [(0, 128), (128, 128), (256, 128), (384, 17)]

    with tile.TileContext(nc) as tc:
        with tc.tile_pool(name="const", bufs=1) as const, \
             tc.tile_pool(name="acts", bufs=16) as acts, \
             tc.tile_pool(name="states", bufs=3) as states, \
             tc.tile_pool(name="tmps", bufs=3) as tmps, \
             tc.tile_pool(name="noisep", bufs=3) as noisep, \
             tc.tile_pool(name="psL", bufs=6, space="PSUM") as psL, \
             tc.tile_pool(name="psF", bufs=1, space="PSUM") as psF, \
             tc.tile_pool(name="psR", bufs=1, space="PSUM") as psR, \
             tc.tile_pool(name="dramp", bufs=1, space="DRAM") as dramp:

            def load(ap, p, f, tag, dt=BF16, eng=None):
                t = const.tile([p, f], dt, tag=tag)
                (eng or nc.sync).dma_start(out=t, in_=ap)
                return t

            # ---- weights / constants (DMA'd straight in as bf16) ----
            # treat-MLP inputs + W1 first (they gate step 0); the bulky
            # mid-layer weights ride the gpsimd DMA queue in parallel
            tw = [load(tw_d[i], 1 if i == 0 else U_DIM, U_DIM, f"tw{i}")
                  for i in range(6)]
            tb = [load(tb_d[i], U_DIM, 1, f"tb{i}", F32) for i in range(6)]
            trow = load(trow_d, 1, n_steps, "trow")
            w1u = load(w1u_d, U_DIM, HID, "w1u")
            # W1 augmented: rows 0:100 = W1z (static), row 100 = V[s] per step
            w1a = const.tile([LATENT + 1, HID], BF16, tag="w1a")
            nc.sync.dma_start(out=w1a[0:LATENT, :], in_=w1z_d)
            b1 = [load(b1_d[m0:m0 + mc, :], mc, 1, f"b1_{mi}", F32)
                  for mi, (m0, mc) in enumerate(CH)]
            # logqp reduction vector in f32r
            onesred_st = const.tile([LATENT, 1], F32, tag="onesred_st")
            nc.sync.dma_start(out=onesred_st, in_=onesred_d)
            onesred = const.tile([LATENT, 1], F32R, tag="onesred")
            nc.vector.tensor_copy(onesred, onesred_st)
            wmid = [[load(wd[k0:k0 + kc, :], kc, HID, f"w{l}_{ki}",
                          eng=nc.gpsimd)
                     for ki, (k0, kc) in enumerate(CH)]
                    for l, wd in zip((2, 3, 4, 5), wmid_d)]
            bmid = [[load(bd[m0:m0 + mc, :], mc, 1, f"b{l}_{mi}", F32,
                          eng=nc.gpsimd)
                     for mi, (m0, mc) in enumerate(CH)]
                    for l, bd in zip((2, 3, 4, 5), bmid_d)]
            w6 = [load(w6a_d[k0:k0 + kc, :], kc, LATENT, f"w6_{ki}",
                       eng=nc.gpsimd)
                  for ki, (k0, kc) in enumerate(CH6)]

            # ---- treat MLP for all steps at once: h [20, n_steps] ----
            h = trow
            for i in range(6):
                p = psR.tile([U_DIM, n_steps], F32, tag="pr")
                nc.tensor.matmul(p, tw[i], h, start=True, stop=True)
                hn = const.tile([U_DIM, n_steps], BF16, tag=f"th{i}")
                nc.scalar.activation(hn, p, Act.Relu if i < 5 else Act.Identity,
                                     bias=tb[i])
                h = hn

            # V[s,:] = u_all[:,s] @ W1u  -> [n_steps, HID] rows, then bounce
            # through DRAM so each V[s] can be DMA'd onto w1a row 100.
            pV = psL.tile([n_steps, HID], F32, tag="pl")
            nc.tensor.matmul(pV, h, w1u, start=True, stop=True)
            vsb = const.tile([n_steps, HID], BF16, tag="vsb")
            nc.scalar.copy(vsb, pV)
            vdram = dramp.tile([n_steps, HID], BF16)
            nc.sync.dma_start(out=vdram, in_=vsb)

            # ---- initial state ----
            z = states.tile([LATENT, ROWS], F32, tag="z")
            nc.sync.dma_start(out=z, in_=z0f)
            # per-step logqp increments parked here, one DMA at the end
            redall = const.tile([1, n_steps * ROWS], F32, tag="redall")
            # fixed bf16 MLP-input tile: rows 0:100 = z (rewritten per
            # step), row 100 = Tcol (written once)
            za = const.tile([LATENT + 1, ROWS], BF16, tag="za")
            nc.sync.dma_start(out=za[LATENT:LATENT + 1, :], in_=tcol_d)
            nc.vector.tensor_copy(za[0:LATENT, :], z)
            # fixed last-chunk tile of act5: row 16 = ones (carries b6)
            act5c3 = const.tile([17, ROWS], BF16, tag="act5c3")
            nc.sync.dma_start(out=act5c3[16:17, :], in_=onesrow_d)

            # ---- Euler scan ----
            for s in range(n_steps):
                nt = noisep.tile([LATENT, ROWS], F32, tag="noise")
                nc.sync.dma_start(out=nt, in_=nzd[s])
                # V[s] onto the augmented-W1 contraction row
                nc.sync.dma_start(out=w1a[LATENT:LATENT + 1, :],
                                  in_=vdram[s:s + 1, :])

                # layer 1: relu(W1z.T@z + V[s] x Tcol + b1)
                prev = []
                for mi, (m0, mc) in enumerate(CH):
                    p = psL.tile([mc, ROWS], F32, tag="pl")
                    nc.tensor.matmul(p, w1a[:, m0:m0 + mc], za,
                                     start=True, stop=True)
                    t = acts.tile([mc, ROWS], BF16, tag="act")
                    nc.vector.tensor_scalar(t, p, b1[mi], 0.0, ADD, MAX)
                    prev.append(t)
                # off-critical-path pieces of the state update:
                # t3 = sig*sqrt(dt)*dW + 1.005*z (ready long before drift)
                s1 = tmps.tile([LATENT, ROWS], F32, tag="s1")
                nc.vector.scalar_tensor_tensor(s1, z, 0.005, z, MULT, ADD)
                t3 = tmps.tile([LATENT, ROWS], F32, tag="t3")
                nc.vector.scalar_tensor_tensor(t3, nt, SQDT_SIG, s1,
                                               MULT, ADD)

                # layers 2-5: tanh(W.T @ x + b)
                for li in range(4):
                    last = li == 3
                    cur = []
                    for mi, (m0, mc) in enumerate(CH):
                        p = psL.tile([mc, ROWS], F32, tag="pl")
                        for ki, (k0, kc) in enumerate(CH):
                            nc.tensor.matmul(p, wmid[li][ki][:, m0:m0 + mc],
                                             prev[ki], start=(ki == 0),
                                             stop=(ki == 3))
                        # last chunk of act5 lives in the fixed ones-row tile
                        t = act5c3 if (last and mi == 3) else \
                            acts.tile([mc, ROWS], BF16, tag="act")
                        nc.scalar.activation(t[0:mc, :], p, Act.Tanh,
                                             bias=bmid[li][mi])
                        cur.append(t)
                    prev = cur

                # layer 6 (+b6 via ones row): pf = drift
                pf = psF.tile([LATENT, ROWS], F32, tag="pf")
                for ki in range(4):
                    nc.tensor.matmul(pf, w6[ki], prev[ki],
                                     start=(ki == 0), stop=(ki == 3))

                # z_new = t3 + 0.05 drift; bf16 copy (za) first since
                # next-step layer 1 waits only on it
                nc.vector.scalar_tensor_tensor(za[0:LATENT, :], pf, DT, t3,
                                               MULT, ADD)
                # q = drift + 0.2 z ; sq = q^2 (ACT) ; logqp increment
                q = tmps.tile([LATENT, ROWS], F32, tag="q")
                nc.vector.scalar_tensor_tensor(q, z, 0.2, pf, MULT, ADD)
                sq = tmps.tile([LATENT, ROWS], F32R, tag="sq")
                nc.scalar.square(sq, q)
                pr = psR.tile([1, ROWS], F32, tag="pr")
                nc.tensor.matmul(pr, onesred, sq, start=True, stop=True)
                nc.vector.tensor_copy(redall[:, s * ROWS:(s + 1) * ROWS], pr)
                znew = states.tile([LATENT, ROWS], F32, tag="z")
                nc.vector.scalar_tensor_tensor(znew, pf, DT, t3, MULT, ADD)
                nc.gpsimd.dma_start(out=traj[s, :, :], in_=znew)
                z = znew

            # ship all logqp increments out in one DMA
            nc.sync.dma_start(
                out=lq_red.rearrange("s r -> (s r)")[None, :], in_=redall)

    nc.compile()
    return nc


def _bf(x):
    return np.ascontiguousarray(x.astype(ml_dtypes.bfloat16))


def _prep_inputs(z0, t, Tx, noise, treat_params, drift_params):
    z0 = np.asarray(z0, np.float32)
    t = np.asarray(t, np.float32)
    Tx = np.asarray(Tx, np.float32)
    noise = np.asarray(noise, np.float32)
    tp = [(np.asarray(W, np.float32), np.asarray(b, np.float32))
          for W, b in treat_params]
    dp = [(np.asarray(W, np.float32), np.asarray(b, np.float32))
          for W, b in drift_params]

    z0_flat = np.ascontiguousarray(z0.reshape(-1, LATENT))  # [4096, 100]
    w6a = np.concatenate([dp[5][0], dp[5][1][None, :]], axis=0)  # [401, 100]
    shared = {
        "trow": _bf(t[:N_STEPS][None, :]),
        "onesred": np.full((LATENT, 1), 0.1, np.float32),
        "onesrow": _bf(np.ones((1, ROWS), np.float32)),
        "w1z": _bf(dp[0][0][:LATENT]),
        "w1u": _bf(dp[0][0][LATENT:]),
        "b1": np.ascontiguousarray(dp[0][1][:, None]),
        "w6a": _bf(w6a),
    }
    for i, l in enumerate((2, 3, 4, 5)):
        shared[f"w{l}"] = _bf(dp[i + 1][0])
        shared[f"b{l}"] = np.ascontiguousarray(dp[i + 1][1][:, None])
    for i in range(6):
        shared[f"tw{i}"] = _bf(tp[i][0])
        shared[f"tb{i}"] = np.ascontiguousarray(tp[i][1][:, None])

    in_maps = []
    bpc = BATCH // NCORES  # batches per core
    for c in range(NCORES):
        r0, r1 = c * ROWS, (c + 1) * ROWS
        m = dict(shared)
        m["z0f"] = np.ascontiguousarray(z0_flat[r0:r1].T)
        m["noise"] = np.ascontiguousarray(
            noise[:, r0:r1, :LATENT].transpose(0, 2, 1))
        m["tcol"] = _bf(np.repeat(Tx[c * bpc:(c + 1) * bpc],
                                  NUM_SAMPLES)[None, :])
        in_maps.append(m)
    return in_maps, z0_flat


def kernel(z0, t, Tx, noise, treat_params, drift_params):
    global LAST_EXEC_NS
    if "nc" not in _CACHE:
        _CACHE["nc"] = _build()
    nc = _CACHE["nc"]

    in_maps, z0_flat = _prep_inputs(z0, t, Tx, noise, treat_params,
                                    drift_params)
    try:
        res = bass_utils.run_bass_kernel_spmd(
            nc, in_maps, core_ids=list(range(NCORES)), trace=TRACE)
    except Exception:
        if not TRACE:
            raise
        res = bass_utils.run_bass_kernel_spmd(
            nc, in_maps, core_ids=list(range(NCORES)), trace=False)
    LAST_EXEC_NS = res.exec_time_ns

    # assemble [51, 4096, 101] then raw-reshape like the reference
    ntotal = BATCH * NUM_SAMPLES
    full = np.empty((T_LEN, ntotal, LATENT + 1), np.float32)
    full[0, :, :LATENT] = z0_flat
    full[0, :, LATENT] = 0.0
    for c in range(NCORES):
        r0, r1 = c * ROWS, (c + 1) * ROWS
        tr = res.results[c]["traj"]  # [n_steps, 100, 512]
        full[1:, r0:r1, :LATENT] = tr.transpose(0, 2, 1)
        # logqp state at time t = cumsum of the per-step increments
        red = res.results[c]["lq_red"]  # [n_steps, 512]
        full[1:, r0:r1, LATENT] = np.cumsum(red, axis=0)
    arr4 = full.reshape(BATCH, NUM_SAMPLES, T_LEN, LATENT + 1)
    ys = np.ascontiguousarray(arr4[:, :, :, :LATENT])
    logqp = np.ascontiguousarray(arr4[:, :, -1, LATENT])
    return ys, logqp
